# revision 1
# baseline (speedup 1.0000x reference)
# Trainium2 Bass kernel for nn_DeformablePatchEmbed_GELU (deformable patch
# embed + BatchNorm(batch stats) + exact GELU), data-parallel over 8 cores.
#
# Algorithm (device side, per core, B_loc=8 images):
#   For each output row ho (14 chunks of 112 positions = 14 wo x 8 b):
#     - DMA the padded 20x20x3 pixel window of every position into SBUF
#       [112 part, 1200 free] (host pre-pads x by 2 -> OOB reads are zeros).
#     - PE-transpose the interior 16x16x3 patch (flat order (c,ki,kj)) to
#       patchT tiles [128, 112] and matmul (fp32r) with the offset-conv
#       weight [768,512] -> offsets [112, 512] = [dy(256) | dx(256)].
#     - Bilinear sampling is decomposed over integer taps s in [-2,2]^2 with
#       hat weights: sampled = sum_s Hat(dy-sy)*Hat(dx-sx) * x[base+s],
#       Hat(u) = relu(1-|u|).  Hats built on ScalarE (Abs,Relu), the 25
#       masked MACs stream on VectorE against strided views of the window.
#     - PE-transpose sampled [112,768] -> matmul (fp32r) with dconv weight
#       (flat order (ki,kj,c)) -> y [112, 768]; stash in SBUF.
#     - BN partial sums: ones-vector matmuls give sum(y), sum(y^2) per o.
#   AllReduce (8 cores) of the 1536 partial sums -> global BN stats ->
#   per-o scale/shift; GELU on ScalarE (exact-erf LUT); DMA out [pos, o].
import numpy as np

import concourse.bacc as bacc
import concourse.bass as bass
import concourse.tile as tile
from concourse import mybir
from concourse.bass_utils import run_bass_kernel_spmd

F32 = mybir.dt.float32
F32R = mybir.dt.float32r
AF = mybir.ActivationFunctionType

# problem dims (hardcoded per contract)
B, C, H, W = 64, 3, 224, 224
O = 768
PATCH = 16
NCORES = 8
BL = B // NCORES            # 8 images per core
HO = WO = 14
PCH = WO * BL               # 112 positions per chunk (wo major, b minor)
NCHUNK = HO                 # 14
PAD = 2
HP = H + 2 * PAD            # 228
J = 768                     # patch flat size
NTOT = float(B * HO * WO)   # 12544 positions globally (BN denominator)
EPS = 1e-5
WIN = 20                    # window side
WROW = WIN * C              # 60
NWIN = WIN * WIN * C        # 1200

_CACHE = {}


def _mkap(handle_ap, offset, dims):
    return bass.AP(tensor=handle_ap.tensor, offset=offset, ap=[list(d) for d in dims])


def _build(n_cores=NCORES):
    nc = bacc.Bacc("TRN2", target_bir_lowering=False, debug=False, num_devices=n_cores)
    xwin = nc.dram_tensor("xwin", [NCHUNK, PCH, NWIN], F32, kind="ExternalInput")
    woff = nc.dram_tensor("woff", [J, 512], F32, kind="ExternalInput")
    wdm = nc.dram_tensor("wdm", [J, O], F32, kind="ExternalInput")
    offb = nc.dram_tensor("offb", [512], F32, kind="ExternalInput")
    bng = nc.dram_tensor("bng", [O], F32, kind="ExternalInput")
    bnb = nc.dram_tensor("bnb", [O], F32, kind="ExternalInput")
    ident = nc.dram_tensor("ident", [PCH, PCH], F32, kind="ExternalInput")
    outd = nc.dram_tensor("out", [BL, HO * WO, O], F32, kind="ExternalOutput")

    outd_b = outd[:]

    from contextlib import ExitStack
    with tile.TileContext(nc) as tc:
        with ExitStack() as ctx:
            consts = ctx.enter_context(tc.tile_pool(name="consts", bufs=1))
            wpool = ctx.enter_context(tc.tile_pool(name="wpool", bufs=2))
            ptpool = ctx.enter_context(tc.tile_pool(name="ptpool", bufs=2))
            dpool = ctx.enter_context(tc.tile_pool(name="dpool", bufs=2))
            hpool = ctx.enter_context(tc.tile_pool(name="hpool", bufs=2))
            mpool = ctx.enter_context(tc.tile_pool(name="mpool", bufs=3))
            tpool = ctx.enter_context(tc.tile_pool(name="tpool", bufs=2))
            apool = ctx.enter_context(tc.tile_pool(name="apool", bufs=2))
            stpool = ctx.enter_context(tc.tile_pool(name="stpool", bufs=2))
            ypool = ctx.enter_context(tc.tile_pool(name="ypool", bufs=NCHUNK))
            sqpool = ctx.enter_context(tc.tile_pool(name="sqpool", bufs=1))
            cpool = ctx.enter_context(tc.tile_pool(name="cpool", bufs=2))
            fpool = ctx.enter_context(tc.tile_pool(name="fpool", bufs=1))
            ps_t = ctx.enter_context(tc.tile_pool(name="ps_t", bufs=2, space="PSUM"))
            ps_off = ctx.enter_context(tc.tile_pool(name="ps_off", bufs=1, space="PSUM"))
            ps_y = ctx.enter_context(tc.tile_pool(name="ps_y", bufs=2, space="PSUM"))
            ps_s = ctx.enter_context(tc.tile_pool(name="ps_s", bufs=1, space="PSUM"))
            drampool = ctx.enter_context(tc.tile_pool(name="dram", bufs=1, space="DRAM"))
            # ---- constants ----
            woff_sb = consts.tile([128, 6, 512], F32)
            nc.sync.dma_start(out=woff_sb, in_=woff[:].rearrange("(t p) n -> p t n", p=128))
            wd_sb = consts.tile([128, 6, O], F32)
            nc.sync.dma_start(out=wd_sb, in_=wdm[:].rearrange("(t p) n -> p t n", p=128))
            ident_sb = consts.tile([PCH, PCH], F32)
            nc.sync.dma_start(out=ident_sb, in_=ident[:])
            ones_sb = consts.tile([PCH, 1], F32)
            nc.vector.memset(ones_sb, 1.0)
            offb_sb = consts.tile([PCH, 512], F32)
            nc.sync.dma_start(out=offb_sb, in_=_mkap(offb[:], 0, [[0, PCH], [1, 512]]))
            sums_sb = consts.tile([1, 1536], F32)
            nc.vector.memset(sums_sb, 0.0)
            # per-partition scalar constants for activation biases
            cbias = {}
            for s in (-2.0, -1.0, 0.0, 1.0, 2.0, EPS):
                cb = consts.tile([128, 1], F32, name=f"cb_{s}")
                nc.vector.memset(cb, float(s))
                cbias[s] = cb

            warm = consts.tile([128, 1], F32, name="warm")
            nc.scalar.activation(warm, cbias[0.0], AF.Erf, bias=cbias[0.0], scale=1.0)

            ystash = []
            # ================= phase A =================
            for ho in range(NCHUNK):
                # window load: partitions are (b, wo) b-major; one DMA per chunk
                wt = wpool.tile([PCH, NWIN], F32, name="wt")
                nc.sync.dma_start(out=wt, in_=xwin[ho])

                # patch in flat order (c, ki, kj), materialized contiguously
                patch = ptpool.tile([PCH, J], F32, name="patch")
                isrc = bass.AP(
                    tensor=wt.tensor, offset=wt.offset + PAD * WROW + PAD * C,
                    ap=[list(wt.ap[0]), [1, C], [WROW, 16], [C, 16]],
                )
                nc.scalar.copy(
                    out=patch.rearrange("p (c ki kj) -> p c ki kj", c=C, ki=16),
                    in_=isrc,
                )
                # patchT tiles via PE transpose of contiguous 128-slices
                ptT = ptpool.tile([128, 6, PCH], F32, name="ptT")
                for t in range(6):
                    tp = ps_t.tile([128, PCH], F32, name="tp")
                    nc.tensor.transpose(tp, patch[:, bass.ts(t, 128)], ident_sb)
                    nc.scalar.copy(out=ptT[:, t, :], in_=tp)

                # offsets matmul: out [112, 512]
                offp = ps_off.tile([PCH, 512], F32, name="offp")
                for t in range(6):
                    nc.tensor.matmul(
                        offp, lhsT=ptT[:, t, :],
                        rhs=woff_sb[:, t, :],
                        start=(t == 0), stop=(t == 5),
                    )
                dyx = dpool.tile([PCH, 512], F32, name="dyx")
                nc.vector.tensor_add(dyx, offp, offb_sb)

                # hats: lam[:, i, :] i in 0..4 -> y taps, 5..9 -> x taps
                lam = hpool.tile([PCH, 10, 256], F32, name="lam")
                for i, s in enumerate((-2, -1, 0, 1, 2)):
                    aby = hpool.tile([PCH, 256], F32, name="aby")
                    nc.scalar.activation(aby, dyx[:, 0:256], AF.Abs, bias=cbias[float(-s)][:PCH], scale=1.0)
                    nc.scalar.activation(lam[:, i, :], aby, AF.Relu, bias=cbias[1.0][:PCH], scale=-1.0)
                    abx = hpool.tile([PCH, 256], F32, name="abx")
                    nc.scalar.activation(abx, dyx[:, 256:512], AF.Abs, bias=cbias[float(-s)][:PCH], scale=1.0)
                    nc.scalar.activation(lam[:, 5 + i, :], abx, AF.Relu, bias=cbias[1.0][:PCH], scale=-1.0)

                # tap MAC: acc[p, ki, kj, c] += m[p,ki,kj] * win[p, ki+2+sy, kj+2+sx, c]
                # Two independent accumulator chains so VectorE and GpSimd
                # stream taps concurrently (GpSimd ~2x slower per op -> 8/25).
                acc = apool.tile([PCH, 768], F32, name="acc")
                accv = acc.rearrange("p (ki kj c) -> p ki kj c", ki=16, kj=16)
                accp = apool.tile([PCH, 768], F32, name="accp")
                accpv = accp.rearrange("p (ki kj c) -> p ki kj c", ki=16, kj=16)
                first_v = True
                first_p = True
                tapi = 0
                for iy in range(5):
                    sy = iy - 2
                    for ix in range(5):
                        sx = ix - 2
                        on_pool = tapi < 9
                        tapi += 1
                        eng = nc.gpsimd if on_pool else nc.vector
                        m = mpool.tile([PCH, 256], F32,
                                       name="mp" if on_pool else "m")
                        eng.tensor_mul(m, lam[:, iy, :], lam[:, 5 + ix, :])
                        mB = (
                            m.rearrange("p (ki kj) -> p ki kj", ki=16)
                            .unsqueeze(-1).broadcast_to([PCH, 16, 16, C])
                        )
                        xoff = (PAD + sy) * WROW + (PAD + sx) * C
                        xs = bass.AP(
                            tensor=wt.tensor, offset=wt.offset + xoff,
                            ap=[list(wt.ap[0]), [WROW, 16], [C, 16], [1, C]],
                        )
                        if on_pool:
                            if first_p:
                                nc.gpsimd.tensor_mul(accpv, xs, mB)
                                first_p = False
                            else:
                                tmp = tpool.tile([PCH, 768], F32, name="tmpp")
                                tv = tmp.rearrange("p (ki kj c) -> p ki kj c", ki=16, kj=16)
                                nc.gpsimd.tensor_mul(tv, xs, mB)
                                nc.gpsimd.tensor_add(accp, accp, tmp)
                        else:
                            if first_v:
                                nc.vector.tensor_mul(accv, xs, mB)
                                first_v = False
                            else:
                                tmp = tpool.tile([PCH, 768], F32, name="tmp")
                                tv = tmp.rearrange("p (ki kj c) -> p ki kj c", ki=16, kj=16)
                                nc.vector.tensor_mul(tv, xs, mB)
                                nc.vector.tensor_add(acc, acc, tmp)
                nc.vector.tensor_add(acc, acc, accp)

                # sampledT via PE transposes
                sT = stpool.tile([128, 6, PCH], F32, name="sT")
                for t in range(6):
                    tp2 = ps_t.tile([128, PCH], F32, name="tp2")
                    nc.tensor.transpose(tp2, acc[:, bass.ts(t, 128)], ident_sb)
                    nc.scalar.copy(out=sT[:, t, :], in_=tp2)

                # main matmul: y [112, 768]
                y = ypool.tile([PCH, O], F32, name="y")
                for half in range(2):
                    yp = ps_y.tile([PCH, 384], F32, name="yp")
                    for t in range(6):
                        nc.tensor.matmul(
                            yp, lhsT=sT[:, t, :],
                            rhs=wd_sb[:, t, bass.ts(half, 384)],
                            start=(t == 0), stop=(t == 5),
                        )
                    nc.scalar.copy(out=y[:, bass.ts(half, 384)], in_=yp)
                ystash.append(y)

                # BN partial sums
                ysq = sqpool.tile([PCH, O], F32, name="ysq")
                nc.scalar.activation(ysq, y, AF.Square, bias=cbias[0.0][:PCH], scale=1.0)
                for seg in range(4):
                    srcseg = (y if seg < 2 else ysq)[:, bass.ts(seg % 2, 384)]
                    sp = ps_s.tile([1, 384], F32, name="sp")
                    nc.tensor.matmul(sp, lhsT=ones_sb, rhs=srcseg,
                                     start=True, stop=True)
                    nc.vector.tensor_add(
                        sums_sb[:, bass.ts(seg, 384)], sums_sb[:, bass.ts(seg, 384)], sp
                    )

            # ================= phase B: global BN stats =================
            cc_in = drampool.tile([1, 1536], F32, name="cc_in")
            cc_out = drampool.tile([1, 1536], F32, name="cc_out", addr_space="Shared")
            nc.sync.dma_start(out=cc_in, in_=sums_sb)
            nc.gpsimd.collective_compute(
                "AllReduce", mybir.AluOpType.add,
                replica_groups=[list(range(n_cores))],
                ins=[cc_in.opt()], outs=[cc_out.opt()],
            )
            gsums = fpool.tile([128, 1536], F32)
            nc.sync.dma_start(out=gsums, in_=_mkap(cc_out, cc_out.offset, [[0, 128], [1, 1536]]))
            asc = fpool.tile([128, O], F32, name="asc")
            bsh = fpool.tile([128, O], F32, name="bsh")

            mean = fpool.tile([128, O], F32, name="ftmp", tag="ftmp", bufs=3)
            nc.scalar.mul(mean, gsums[:, 0:768], 1.0 / NTOT)
            var = fpool.tile([128, O], F32, name="ftmp2", tag="ftmp", bufs=3)
            nc.vector.tensor_mul(var, mean, mean)
            # var = E[y^2]/N - mean^2  (in place on var)
            nc.scalar.mul(gsums[:, 768:1536], gsums[:, 768:1536], 1.0 / NTOT)
            nc.vector.tensor_sub(var, gsums[:, 768:1536], var)
            # rstd = rsqrt(var + eps) via bit-trick + 3 Newton steps (DVE only)
            vpe = fpool.tile([128, O], F32, name="ftmp3", tag="ftmp", bufs=3)
            nc.vector.tensor_scalar_add(vpe, var, EPS)
            rstd = fpool.tile([128, O], F32, name="ftmp4", tag="ftmp", bufs=3)
            half_i = fpool.tile([128, O], mybir.dt.int32, name="half_i", tag="ftmpn", bufs=2)
            nc.vector.tensor_scalar(
                half_i, vpe.bitcast(mybir.dt.int32), 1, None,
                mybir.AluOpType.arith_shift_right,
            )
            nc.vector.tensor_scalar(
                half_i, half_i, -1, None, mybir.AluOpType.mult
            )
            nc.vector.tensor_scalar_add(
                rstd.bitcast(mybir.dt.int32), half_i, 0x5F3759DF
            )
            nt = fpool.tile([128, O], F32, name="ftmpn", tag="ftmpn", bufs=2)
            for _ in range(3):
                nc.vector.tensor_mul(nt, vpe, rstd)
                nc.vector.tensor_mul(nt, nt, rstd)
                nc.vector.tensor_scalar(
                    nt, nt, -0.5, 1.5, mybir.AluOpType.mult, mybir.AluOpType.add
                )
                nc.vector.tensor_mul(rstd, rstd, nt)
            # asc = gamma * rstd ; bsh = beta - mean * asc
            gam = fpool.tile([128, O], F32, name="ftmp5", tag="ftmp", bufs=3)
            nc.sync.dma_start(out=gam, in_=_mkap(bng[:], 0, [[0, 128], [1, O]]))
            nc.vector.tensor_mul(asc, gam, rstd)
            bet = fpool.tile([128, O], F32, name="ftmp6", tag="ftmp", bufs=3)
            nc.sync.dma_start(out=bet, in_=_mkap(bnb[:], 0, [[0, 128], [1, O]]))
            nc.vector.tensor_mul(bsh, mean, asc)
            nc.vector.tensor_sub(bsh, bet, bsh)
            # fold gelu's 0.5 into the BN affine: yn2 = 0.5*(y*a+b)
            nc.vector.tensor_scalar_mul(asc, asc, 0.5)
            nc.vector.tensor_scalar_mul(bsh, bsh, 0.5)

            # ================= phase C: normalize + GELU + store =================
            for ho in range(NCHUNK):
                y = ystash[ho]
                yn = cpool.tile([PCH, O], F32, name="yn")
                nc.vector.tensor_mul(yn, y, asc[:PCH, :])
                nc.vector.tensor_add(yn, yn, bsh[:PCH, :])
                g = cpool.tile([PCH, O], F32, name="g")
                # yn here is 0.5*(BN affine); gelu = (erf(yn*2/sqrt2)+1)*yn
                nc.scalar.activation(g, yn, AF.Erf, bias=cbias[0.0][:PCH],
                                     scale=1.4142135623730951)
                nc.vector.scalar_tensor_tensor(
                    g, g, 1.0, yn, mybir.AluOpType.add, mybir.AluOpType.mult
                )
                for bb in range(BL):
                    nc.sync.dma_start(
                        out=outd_b[bb, ho * WO:(ho + 1) * WO, :],
                        in_=g[bb * WO:(bb + 1) * WO, :],
                    )

    nc.compile()
    return nc


def _host_prep(x, offset_w, offset_b, dconv_w):
    xt = np.transpose(np.asarray(x, np.float32), (0, 2, 3, 1))
    xpad = np.zeros((B, HP, HP, C), np.float32)
    xpad[:, PAD:PAD + H, PAD:PAD + W, :] = xt
    # windows with halo: [B, ho, wo, 20, 20, 3] -> per-chunk layout
    sb, sy, sx, sc = xpad.strides
    win = np.lib.stride_tricks.as_strided(
        xpad, shape=(B, HO, WO, WIN, WIN, C),
        strides=(sb, 16 * sy, 16 * sx, sy, sx, sc),
    )
    # [ho, b, wo, win] per full batch
    xwin = np.ascontiguousarray(win.transpose(1, 0, 2, 3, 4, 5)).reshape(
        HO, B, WO, NWIN
    )
    woff = np.asarray(offset_w, np.float32).transpose(1, 2, 3, 0).reshape(J, 512)
    perm = np.r_[np.arange(0, 512, 2), np.arange(1, 512, 2)]
    woff = np.ascontiguousarray(woff[:, perm])
    offbp = np.ascontiguousarray(np.asarray(offset_b, np.float32)[perm])
    wd = np.ascontiguousarray(
        np.asarray(dconv_w, np.float32).transpose(2, 3, 1, 0).reshape(J, O)
    )
    return xwin, woff, offbp, wd


def kernel(x, offset_w, offset_b, dconv_w, bn_gamma, bn_beta):
    if "nc" not in _CACHE:
        _CACHE["nc"] = _build()
    nc = _CACHE["nc"]

    xwin, woff, offbp, wd = _host_prep(x, offset_w, offset_b, dconv_w)
    ident = np.eye(PCH, dtype=np.float32)
    bng = np.asarray(bn_gamma, np.float32)
    bnb = np.asarray(bn_beta, np.float32)

    in_maps = []
    for c in range(NCORES):
        in_maps.append({
            "xwin": np.ascontiguousarray(
                xwin[:, c * BL:(c + 1) * BL].reshape(NCHUNK, PCH, NWIN)),
            "woff": woff, "wdm": wd, "offb": offbp,
            "bng": bng, "bnb": bnb, "ident": ident,
        })
    res = run_bass_kernel_spmd(nc, in_maps, list(range(NCORES)))
    outs = [res.results[c]["out"] for c in range(NCORES)]
    return np.concatenate(outs, axis=0).astype(np.float32)


if __name__ == "__main__":
    # smoke: build only
    _build()
    print("build ok")



# revision 15
# speedup vs baseline: 1.0953x; 1.0953x over previous
# Trainium2 Bass kernel for nn_DeformablePatchEmbed_GELU (deformable patch
# embed + BatchNorm(batch stats) + exact GELU), data-parallel over 8 cores.
#
# v2: bf16 datapath. Per core, 1568 positions (8 images x 14x14) packed as
# 13 chunks of 128 partition-rows (last chunk 32 real rows, zero-padded).
# Windows are stored c-major [c, wi, wj] (20x20x3) in bf16 so every DVE
# tensor op has a packed (stride-1) innermost dim -> 2x DVE rate, and the
# PE matmuls/transposes run 1-pass bf16 (~5x faster than fp32).
#
# Per chunk:
#   - one DMA loads win [128, 1200] bf16
#   - PE transposes 6 strided 128-column slices of the interior patch
#     (flat (c,ki,kj)) -> patchT; matmul with offset weights -> offsets
#   - hats Hat(u)=relu(1-|u|) on ScalarE (per-partition bias = -s)
#   - one DVE op forms all 25 tap products m2[sy,sx,k]=haty*hatx
#   - bilinear = sum_s m2_s * win_shift_s: mul+add chains split between
#     VectorE (bf16 2x) and GpSimd, two independent accumulators
#   - PE transposes acc -> sampledT; matmul with dconv weights -> y
#   - BN partial sums via ones-matmuls, accumulated into SBUF
# AllReduce (8 cores) of 1536 sums -> BN scale/shift (folded w/ GELU 0.5);
# phase C: normalize + exact GELU (Erf LUT) + store fp32.
import numpy as np
import ml_dtypes

import concourse.bacc as bacc
import concourse.bass as bass
import concourse.tile as tile
from concourse import mybir
from concourse.bass_utils import run_bass_kernel_spmd

F32 = mybir.dt.float32
BF16 = mybir.dt.bfloat16
AF = mybir.ActivationFunctionType
BF = ml_dtypes.bfloat16

# problem dims (hardcoded per contract)
B, C, H, W = 64, 3, 224, 224
O = 768
PATCH = 16
NCORES = 8
BL = B // NCORES            # 8 images per core
HO = WO = 14
NPOS = BL * HO * WO         # 1568 positions per core
PCH = 128                   # positions per chunk (partition rows)
NCHUNK = 13                 # ceil(1568/128); last chunk has 32 real rows
NPAD = NCHUNK * PCH         # 1664
PAD = 2
J = 768                     # patch flat size (c,ki,kj)
NTOT = float(B * HO * WO)   # 12544 positions globally (BN denominator)
EPS = 1e-5
WIN = 20                    # window side
NWIN = WIN * WIN * C        # 1200, stored c-major: idx = c*400 + wi*20 + wj
SQRT2 = 1.4142135623730951

# tap split: first TD taps on DVE, rest on GpSimd(Pool)
TAPS = [(sy, sx) for sy in range(-2, 3) for sx in range(-2, 3)
        if not (abs(sy) == 2 and abs(sx) == 2)]
TD = 11

_CACHE = {}


def _mkap(handle_ap, offset, dims):
    return bass.AP(tensor=handle_ap.tensor, offset=offset, ap=[list(d) for d in dims])


def _build(n_cores=NCORES):
    nc = bacc.Bacc("TRN2", target_bir_lowering=False, debug=False, num_devices=n_cores)
    xwin = nc.dram_tensor("xwin", [NCHUNK, PCH, NWIN], BF16, kind="ExternalInput")
    woff = nc.dram_tensor("woff", [J, 512], BF16, kind="ExternalInput")
    wdm = nc.dram_tensor("wdm", [J, O], BF16, kind="ExternalInput")
    offb = nc.dram_tensor("offb", [512], BF16, kind="ExternalInput")
    bng = nc.dram_tensor("bng", [O], F32, kind="ExternalInput")
    bnb = nc.dram_tensor("bnb", [O], F32, kind="ExternalInput")
    ident = nc.dram_tensor("ident", [128, 128], BF16, kind="ExternalInput")
    outd = nc.dram_tensor("out", [NPAD, O], F32, kind="ExternalOutput")

    from contextlib import ExitStack
    with tile.TileContext(nc) as tc:
        with ExitStack() as ctx:
            consts = ctx.enter_context(tc.tile_pool(name="consts", bufs=1))
            wpool = ctx.enter_context(tc.tile_pool(name="wpool", bufs=3))
            ptpool = ctx.enter_context(tc.tile_pool(name="ptpool", bufs=2))
            dpool = ctx.enter_context(tc.tile_pool(name="dpool", bufs=2))
            lpool = ctx.enter_context(tc.tile_pool(name="lpool", bufs=2))
            mpool = ctx.enter_context(tc.tile_pool(name="mpool", bufs=2))
            apool = ctx.enter_context(tc.tile_pool(name="apool", bufs=2))
            tpool = ctx.enter_context(tc.tile_pool(name="tpool", bufs=2))
            stpool = ctx.enter_context(tc.tile_pool(name="stpool", bufs=2))
            ypool = ctx.enter_context(tc.tile_pool(name="ypool", bufs=NCHUNK))
            sqpool = ctx.enter_context(tc.tile_pool(name="sqpool", bufs=2))
            cpool = ctx.enter_context(tc.tile_pool(name="cpool", bufs=4))
            gpool = ctx.enter_context(tc.tile_pool(name="gpool", bufs=4))
            fpool = ctx.enter_context(tc.tile_pool(name="fpool", bufs=1))
            ps_t = ctx.enter_context(tc.tile_pool(name="ps_t", bufs=2, space="PSUM"))
            ps_off = ctx.enter_context(tc.tile_pool(name="ps_off", bufs=1, space="PSUM"))
            ps_y = ctx.enter_context(tc.tile_pool(name="ps_y", bufs=1, space="PSUM"))
            ps_s = ctx.enter_context(tc.tile_pool(name="ps_s", bufs=1, space="PSUM"))
            drampool = ctx.enter_context(tc.tile_pool(name="dram", bufs=1, space="DRAM"))

            # ---- constants (ordered so chunk-0 work starts early) ----
            ident_sb = consts.tile([128, 128], BF16)
            nc.sync.dma_start(out=ident_sb, in_=ident[:])

            wts = {}

            def load_wt(t):
                w = wpool.tile([PCH, NWIN], BF16, name="wt")
                nc.sync.dma_start(out=w, in_=xwin[t])
                wts[t] = w

            load_wt(0)
            woff_sb = consts.tile([128, 6, 512], BF16)
            nc.sync.dma_start(out=woff_sb, in_=woff[:].rearrange("(t p) n -> p t n", p=128))
            offb_sb = consts.tile([128, 512], BF16)
            nc.sync.dma_start(out=offb_sb, in_=_mkap(offb[:], 0, [[0, 128], [1, 512]]))
            load_wt(1)
            wd_sb = consts.tile([128, 6, O], BF16)
            nc.sync.dma_start(out=wd_sb, in_=wdm[:].rearrange("(t p) n -> p t n", p=128))
            ones_sb = consts.tile([128, 1], BF16)
            nc.vector.memset(ones_sb, 1.0)
            sums_sb = consts.tile([1, 1536], F32)
            # per-partition scalar constants for activation biases
            cbias = {}
            for s in (-2.0, -1.0, 0.0, 1.0, 2.0, EPS):
                cb = consts.tile([128, 1], F32, name=f"cb_{s}")
                nc.vector.memset(cb, float(s))
                cbias[s] = cb
            # warm the activation table set containing Erf (+Abs/Relu/Square)
            warm = consts.tile([128, 1], F32, name="warm")
            nc.scalar.activation(warm, cbias[0.0], AF.Erf, bias=cbias[0.0], scale=1.0)

            # BN partial sums accumulate in PSUM across all chunks
            sums_ps = ps_s.tile([1, 2048], F32, name="sums_ps")

            lams = {}
            m2s = {}
            offps = {}

            def front_end(t):
                # PE transposes of strided interior views + offsets matmul +
                # dyx (Pool) + hats (ScalarE). No DVE ops here.
                wt = wts[t]
                # contiguous (c,ki,kj) patch: the PE transpose ifmap must be
                # a single-free-dim AP, so copy the strided interior first
                patch = ptpool.tile([PCH, J], BF16, name="patch")
                isrc = _mkap(
                    wt, wt.offset + PAD * WIN + PAD,
                    [list(wt.ap[0]), [400, C], [WIN, 16], [1, 16]],
                )
                nc.scalar.copy(
                    out=patch.rearrange("p (c ki kj) -> p c ki kj", c=C, ki=16),
                    in_=isrc,
                )
                ptT = ptpool.tile([128, 6, PCH], BF16, name="ptT")
                for q in range(6):
                    tp = ps_t.tile([128, PCH], BF16, name="tp")
                    nc.tensor.transpose(tp, patch[:, bass.ts(q, 128)], ident_sb)
                    nc.scalar.copy(out=ptT[:, q, :], in_=tp)
                offp = ps_off.tile([PCH, 512], F32, name="offp")
                for q in range(6):
                    nc.tensor.matmul(
                        offp, lhsT=ptT[:, q, :], rhs=woff_sb[:, q, :],
                        start=(q == 0), stop=(q == 5),
                    )
                offps[t] = offp

            def mid_end(t):
                # dyx on DVE (GpSimd cannot read PSUM on HW) + hats on ScalarE
                dyx = dpool.tile([PCH, 512], BF16, name="dyx")
                nc.vector.tensor_add(dyx, offps.pop(t), offb_sb)
                lam = lpool.tile([PCH, 5, 512], BF16, name="lam")
                for i, s in enumerate((-2, -1, 0, 1, 2)):
                    ab = lpool.tile([PCH, 512], BF16, name="ab")
                    nc.scalar.activation(ab, dyx, AF.Abs,
                                         bias=cbias[float(-s)], scale=1.0)
                    nc.scalar.activation(lam[:, i, :], ab, AF.Relu,
                                         bias=cbias[1.0], scale=-1.0)
                lams[t] = lam

            def emit_m2d(t):
                # m2[p, sy, sx, k] = lam_y[p, sy, k] * lam_x[p, sx, k],
                # sy rows 0..2 (DVE half)
                lam = lams[t]
                m2 = m2s[t] = mpool.tile([PCH, 25, 256], BF16, name="m2")
                m2o = _mkap(m2, m2.offset, [list(m2.ap[0]), [1280, 3], [256, 5], [1, 256]])
                lyv = _mkap(lam, lam.offset, [list(lam.ap[0]), [512, 3], [0, 5], [1, 256]])
                lxv = _mkap(lam, lam.offset + 256, [list(lam.ap[0]), [0, 3], [512, 5], [1, 256]])
                nc.vector.tensor_mul(m2o, lyv, lxv)

            def emit_m2p(t):
                # sy rows 3..4 (Pool half)
                lam, m2 = lams[t], m2s[t]
                m2o = _mkap(m2, m2.offset + 3 * 1280, [list(m2.ap[0]), [1280, 2], [256, 5], [1, 256]])
                lyv = _mkap(lam, lam.offset + 3 * 512, [list(lam.ap[0]), [512, 2], [0, 5], [1, 256]])
                lxv = _mkap(lam, lam.offset + 256, [list(lam.ap[0]), [0, 2], [512, 5], [1, 256]])
                nc.gpsimd.tensor_mul(m2o, lyv, lxv)

            TAPS_D = [tap for tap in TAPS if tap[0] <= 0]   # 13 taps
            TAPS_P = [tap for tap in TAPS if tap[0] > 0]    # 8 taps

            front_end(0)
            mid_end(0)
            front_end(1)
            mid_end(1)
            emit_m2d(0)
            emit_m2p(0)

            ystash = []
            # ================= phase A (software-pipelined, depth 2) =========
            for t in range(NCHUNK):
                if t + 2 < NCHUNK:
                    load_wt(t + 2)
                    front_end(t + 2)

                wt = wts[t]
                m2 = m2s[t]
                # tap MAC: acc[p,c,ki,kj] += m2_s[p,ki,kj] * win[p,c,ki+2+sy,kj+2+sx]
                accD = apool.tile([PCH, 768], BF16, name="accD")
                accP = apool.tile([PCH, 768], BF16, name="accP")
                for on_d, taps in ((True, TAPS_D), (False, TAPS_P)):
                    eng = nc.vector if on_d else nc.gpsimd
                    acc = accD if on_d else accP
                    av = acc.rearrange("p (c ki kj) -> p c ki kj", c=C, ki=16)
                    for i, (sy, sx) in enumerate(taps):
                        xs = _mkap(
                            wt, wt.offset + (PAD + sy) * WIN + (PAD + sx),
                            [list(wt.ap[0]), [400, C], [WIN, 16], [1, 16]],
                        )
                        mi = (sy + 2) * 5 + (sx + 2)
                        ms = _mkap(
                            m2, m2.offset + mi * 256,
                            [list(m2.ap[0]), [0, C], [16, 16], [1, 16]],
                        )
                        if i == 0:
                            eng.tensor_mul(av, xs, ms)
                        else:
                            tmp = tpool.tile([PCH, 768], BF16,
                                             name="tmpD" if on_d else "tmpP")
                            tv = tmp.rearrange("p (c ki kj) -> p c ki kj", c=C, ki=16)
                            eng.tensor_mul(tv, xs, ms)
                            eng.tensor_add(acc, acc, tmp)
                if t + 1 < NCHUNK:
                    emit_m2d(t + 1)
                    emit_m2p(t + 1)
                nc.vector.tensor_add(accD, accD, accP)

                # sampledT via PE transposes
                sT = stpool.tile([128, 6, PCH], BF16, name="sT")
                for q in range(6):
                    tp2 = ps_t.tile([128, PCH], BF16, name="tp")
                    nc.tensor.transpose(tp2, accD[:, bass.ts(q, 128)], ident_sb)
                    nc.scalar.copy(out=sT[:, q, :], in_=tp2)

                # main matmul: y [128, 768] in two PSUM halves
                y = ypool.tile([PCH, O], BF16, name="y")
                for half in range(2):
                    yp = ps_y.tile([PCH, 384], F32, name="yp")
                    for q in range(6):
                        nc.tensor.matmul(
                            yp, lhsT=sT[:, q, :],
                            rhs=wd_sb[:, q, bass.ts(half, 384)],
                            start=(q == 0), stop=(q == 5),
                        )
                    nc.scalar.copy(out=y[:, bass.ts(half, 384)], in_=yp)
                ystash.append(y)

                # BN partial sums accumulate in PSUM (pad rows have y == 0)
                ysq = sqpool.tile([PCH, O], BF16, name="ysq")
                nc.scalar.activation(ysq, y, AF.Square, bias=cbias[0.0], scale=1.0)
                for seg in range(4):
                    srcseg = (y if seg < 2 else ysq)[:, bass.ts(seg % 2, 384)]
                    nc.tensor.matmul(
                        sums_ps[:, seg * 512: seg * 512 + 384],
                        lhsT=ones_sb, rhs=srcseg,
                        start=(t == 0), stop=(t == NCHUNK - 1),
                    )
                if t + 2 < NCHUNK:
                    mid_end(t + 2)
            nc.scalar.activation(warm, cbias[0.0], AF.Sqrt, bias=cbias[EPS], scale=1.0)
            sums_v = _mkap(sums_ps, sums_ps.offset, [list(sums_ps.ap[0]), [512, 4], [1, 384]])
            nc.scalar.copy(
                out=_mkap(sums_sb, sums_sb.offset, [list(sums_sb.ap[0]), [384, 4], [1, 384]]),
                in_=sums_v)

            # ================= phase B: global BN stats =================
            cc_in = drampool.tile([1, 1536], F32, name="cc_in")
            cc_out = drampool.tile([1, 1536], F32, name="cc_out", addr_space="Shared")
            nc.sync.dma_start(out=cc_in, in_=sums_sb)
            nc.gpsimd.collective_compute(
                "AllReduce", mybir.AluOpType.add,
                replica_groups=[list(range(n_cores))],
                ins=[cc_in.opt()], outs=[cc_out.opt()],
            )
            gsums = fpool.tile([128, 1536], F32)
            nc.sync.dma_start(out=gsums, in_=_mkap(cc_out, cc_out.offset, [[0, 128], [1, 1536]]))
            ascb = fpool.tile([128, O], BF16, name="ascb")
            bshb = fpool.tile([128, O], BF16, name="bshb")

            def dp_split(fn):
                # run an elementwise [128, 768] step as two half-width ops,
                # DVE on [0:384], Pool on [384:768]
                fn(nc.vector, slice(0, 384))
                fn(nc.gpsimd, slice(384, 768))

            gam = fpool.tile([128, O], F32, name="gam")
            nc.sync.dma_start(out=gam, in_=_mkap(bng[:], 0, [[0, 128], [1, O]]))
            bet = fpool.tile([128, O], F32, name="bet")
            nc.sync.dma_start(out=bet, in_=_mkap(bnb[:], 0, [[0, 128], [1, O]]))
            mean = fpool.tile([128, O], F32, name="ftmp", tag="ftmp", bufs=3)
            dp_split(lambda e, s: e.tensor_scalar_mul(mean[:, s], gsums[:, 0:768][:, s], 1.0 / NTOT))
            var = fpool.tile([128, O], F32, name="ftmp2", tag="ftmp", bufs=3)
            dp_split(lambda e, s: e.tensor_mul(var[:, s], mean[:, s], mean[:, s]))
            # var = S2/N - mean^2 in one fused op (TensorScalarPtr is DVE-only)
            nc.vector.scalar_tensor_tensor(
                var, gsums[:, 768:1536], 1.0 / NTOT, var,
                mybir.AluOpType.mult, mybir.AluOpType.subtract)
            # rstd = 1/sqrt(var + eps): ScalarE sqrt + fast DVE reciprocal
            sd = fpool.tile([128, O], F32, name="ftmp3", tag="ftmp", bufs=3)
            nc.scalar.activation(sd, var, AF.Sqrt, bias=cbias[EPS], scale=1.0)
            rstd = fpool.tile([128, O], F32, name="ftmp4", tag="ftmp", bufs=3)
            nc.vector.reciprocal_approx_fast(rstd, sd)
            # asc = (gamma/2)*rstd ; bsh = beta/2 - mean*asc (GELU 0.5
            # pre-folded into bng/bnb on the host)
            dp_split(lambda e, s: e.tensor_mul(ascb[:, s], gam[:, s], rstd[:, s]))
            bsh = fpool.tile([128, O], F32, name="bsh")
            dp_split(lambda e, s: e.tensor_mul(bsh[:, s], mean[:, s], ascb[:, s]))
            dp_split(lambda e, s: e.tensor_sub(bshb[:, s], bet[:, s], bsh[:, s]))

            # ================= phase C: normalize + GELU + store =================
            for t in range(NCHUNK):
                y = ystash[t]
                ym = cpool.tile([PCH, O], BF16, name="ym")
                nc.vector.tensor_mul(ym, y, ascb)
                yn = cpool.tile([PCH, O], BF16, name="yn")
                nc.gpsimd.tensor_add(yn, ym, bshb)
                g = cpool.tile([PCH, O], BF16, name="g")
                # yn = 0.5*(BN affine); gelu = (erf(yn*2/sqrt2)+1)*yn
                nc.scalar.activation(g, yn, AF.Erf, bias=cbias[0.0], scale=SQRT2)
                gout = gpool.tile([PCH, O], F32, name="gout")
                nc.vector.scalar_tensor_tensor(
                    gout, g, 1.0, yn, mybir.AluOpType.add, mybir.AluOpType.mult
                )
                nrows = min(PCH, NPOS - t * PCH)
                nc.sync.dma_start(
                    out=outd[t * PCH: t * PCH + nrows, :],
                    in_=gout[:nrows, :],
                )

    nc.compile()
    return nc


def _host_prep(x, offset_w, offset_b, dconv_w):
    x = np.asarray(x, np.float32)
    xpad = np.zeros((B, C, H + 2 * PAD, W + 2 * PAD), np.float32)
    xpad[:, :, PAD:PAD + H, PAD:PAD + W] = x
    sb, sc, sy, sx = xpad.strides
    # windows c-major: [B, ho, wo, c, wi, wj]
    win6 = np.lib.stride_tricks.as_strided(
        xpad, shape=(B, HO, WO, C, WIN, WIN),
        strides=(sb, 16 * sy, 16 * sx, sc, sy, sx),
    )
    xwin = np.ascontiguousarray(win6).reshape(B, HO * WO, NWIN).astype(BF)

    # weights to flat-j (c, ki, kj) order
    woff = np.asarray(offset_w, np.float32).transpose(1, 2, 3, 0).reshape(J, 512)
    perm = np.r_[np.arange(0, 512, 2), np.arange(1, 512, 2)]
    woff = np.ascontiguousarray(woff[:, perm]).astype(BF)
    offbp = np.ascontiguousarray(np.asarray(offset_b, np.float32)[perm]).astype(BF)
    wd = np.ascontiguousarray(
        np.asarray(dconv_w, np.float32).transpose(1, 2, 3, 0).reshape(J, O)
    ).astype(BF)
    return xwin, woff, offbp, wd


def _in_maps(x, offset_w, offset_b, dconv_w, bn_gamma, bn_beta):
    xwin, woff, offbp, wd = _host_prep(x, offset_w, offset_b, dconv_w)
    ident = np.eye(128, dtype=BF)
    bngk = 0.5 * np.asarray(bn_gamma, np.float32)
    bnbk = 0.5 * np.asarray(bn_beta, np.float32)
    in_maps = []
    for c in range(NCORES):
        xc = xwin[c * BL:(c + 1) * BL].reshape(NPOS, NWIN)
        xc_pad = np.zeros((NPAD, NWIN), BF)
        xc_pad[:NPOS] = xc
        in_maps.append({
            "xwin": np.ascontiguousarray(xc_pad.reshape(NCHUNK, PCH, NWIN)),
            "woff": woff, "wdm": wd, "offb": offbp,
            "bng": bngk, "bnb": bnbk, "ident": ident,
        })
    return in_maps


def kernel(x, offset_w, offset_b, dconv_w, bn_gamma, bn_beta):
    if "nc" not in _CACHE:
        _CACHE["nc"] = _build()
    nc = _CACHE["nc"]
    in_maps = _in_maps(x, offset_w, offset_b, dconv_w, bn_gamma, bn_beta)
    res = run_bass_kernel_spmd(nc, in_maps, list(range(NCORES)))
    outs = [res.results[c]["out"][:NPOS] for c in range(NCORES)]
    return np.concatenate(outs, axis=0).reshape(B, HO * WO, O).astype(np.float32)


if __name__ == "__main__":
    _build()
    print("build ok")


# revision 26
# speedup vs baseline: 1.1159x; 1.0188x over previous
# Trainium2 Bass kernel for nn_DeformablePatchEmbed_GELU (deformable patch
# embed + BatchNorm(batch stats) + exact GELU), data-parallel over 8 cores.
#
# v2: bf16 datapath. Per core, 1568 positions (8 images x 14x14) packed as
# 13 chunks of 128 partition-rows (last chunk 32 real rows, zero-padded).
# Windows are stored c-major [c, wi, wj] (20x20x3) in bf16 so every DVE
# tensor op has a packed (stride-1) innermost dim -> 2x DVE rate, and the
# PE matmuls/transposes run 1-pass bf16 (~5x faster than fp32).
#
# Per chunk:
#   - one DMA loads win [128, 1200] bf16
#   - PE transposes 6 strided 128-column slices of the interior patch
#     (flat (c,ki,kj)) -> patchT; matmul with offset weights -> offsets
#   - hats Hat(u)=relu(1-|u|) on ScalarE (per-partition bias = -s)
#   - one DVE op forms all 25 tap products m2[sy,sx,k]=haty*hatx
#   - bilinear = sum_s m2_s * win_shift_s: mul+add chains split between
#     VectorE (bf16 2x) and GpSimd, two independent accumulators
#   - PE transposes acc -> sampledT; matmul with dconv weights -> y
#   - BN partial sums via ones-matmuls, accumulated into SBUF
# AllReduce (8 cores) of 1536 sums -> BN scale/shift (folded w/ GELU 0.5);
# phase C: normalize + exact GELU (Erf LUT) + store fp32.
import numpy as np
import ml_dtypes

import concourse.bacc as bacc
import concourse.bass as bass
import concourse.tile as tile
from concourse import mybir
from concourse.bass_utils import run_bass_kernel_spmd

F32 = mybir.dt.float32
BF16 = mybir.dt.bfloat16
AF = mybir.ActivationFunctionType
BF = ml_dtypes.bfloat16

# problem dims (hardcoded per contract)
B, C, H, W = 64, 3, 224, 224
O = 768
PATCH = 16
NCORES = 8
BL = B // NCORES            # 8 images per core
HO = WO = 14
NPOS = BL * HO * WO         # 1568 positions per core
PCH = 128                   # positions per chunk (partition rows)
NCHUNK = 13                 # ceil(1568/128); last chunk has 32 real rows
NPAD = NCHUNK * PCH         # 1664
PAD = 2
J = 768                     # patch flat size (c,ki,kj)
NTOT = float(B * HO * WO)   # 12544 positions globally (BN denominator)
EPS = 1e-5
WIN = 20                    # window side
NWIN = WIN * WIN * C        # 1200, stored c-major: idx = c*400 + wi*20 + wj
SQRT2 = 1.4142135623730951

# tap split: first TD taps on DVE, rest on GpSimd(Pool)
TAPS = [(sy, sx) for sy in range(-2, 3) for sx in range(-2, 3)
        if not (abs(sy) == 2 and abs(sx) == 2)]
TD = 11

_CACHE = {}


def _mkap(handle_ap, offset, dims):
    return bass.AP(tensor=handle_ap.tensor, offset=offset, ap=[list(d) for d in dims])


def _build(n_cores=NCORES):
    nc = bacc.Bacc("TRN2", target_bir_lowering=False, debug=False, num_devices=n_cores)
    xwin = nc.dram_tensor("xwin", [NCHUNK, PCH, NWIN], BF16, kind="ExternalInput")
    woff = nc.dram_tensor("woff", [J, 512], BF16, kind="ExternalInput")
    wdm = nc.dram_tensor("wdm", [J, O], BF16, kind="ExternalInput")
    offb = nc.dram_tensor("offb", [512], BF16, kind="ExternalInput")
    bng = nc.dram_tensor("bng", [O], F32, kind="ExternalInput")
    bnb = nc.dram_tensor("bnb", [O], F32, kind="ExternalInput")
    ident = nc.dram_tensor("ident", [128, 128], BF16, kind="ExternalInput")
    outd = nc.dram_tensor("out", [NPAD, O], F32, kind="ExternalOutput")

    from contextlib import ExitStack
    with tile.TileContext(nc) as tc:
        with ExitStack() as ctx:
            consts = ctx.enter_context(tc.tile_pool(name="consts", bufs=1))
            wpool = ctx.enter_context(tc.tile_pool(name="wpool", bufs=3))
            ptpool = ctx.enter_context(tc.tile_pool(name="ptpool", bufs=2))
            dpool = ctx.enter_context(tc.tile_pool(name="dpool", bufs=2))
            lpool = ctx.enter_context(tc.tile_pool(name="lpool", bufs=2))
            mpool = ctx.enter_context(tc.tile_pool(name="mpool", bufs=2))
            apool = ctx.enter_context(tc.tile_pool(name="apool", bufs=2))
            tpool = ctx.enter_context(tc.tile_pool(name="tpool", bufs=2))
            stpool = ctx.enter_context(tc.tile_pool(name="stpool", bufs=2))
            ypool = ctx.enter_context(tc.tile_pool(name="ypool", bufs=NCHUNK))
            sqpool = ctx.enter_context(tc.tile_pool(name="sqpool", bufs=2))
            cpool = ctx.enter_context(tc.tile_pool(name="cpool", bufs=4))
            gpool = ctx.enter_context(tc.tile_pool(name="gpool", bufs=4))
            fpool = ctx.enter_context(tc.tile_pool(name="fpool", bufs=1))
            ps_t = ctx.enter_context(tc.tile_pool(name="ps_t", bufs=2, space="PSUM"))
            ps_off = ctx.enter_context(tc.tile_pool(name="ps_off", bufs=1, space="PSUM"))
            ps_y = ctx.enter_context(tc.tile_pool(name="ps_y", bufs=1, space="PSUM"))
            ps_s = ctx.enter_context(tc.tile_pool(name="ps_s", bufs=1, space="PSUM"))
            drampool = ctx.enter_context(tc.tile_pool(name="dram", bufs=1, space="DRAM"))

            # ---- constants (ordered so chunk-0 work starts early) ----
            ident_sb = consts.tile([128, 128], BF16)
            nc.sync.dma_start(out=ident_sb, in_=ident[:])

            wts = {}

            def load_wt(t):
                w = wpool.tile([PCH, NWIN], BF16, name="wt")
                nc.sync.dma_start(out=w, in_=xwin[t])
                wts[t] = w

            load_wt(0)
            woff_sb = consts.tile([128, 6, 512], BF16)
            nc.sync.dma_start(out=woff_sb, in_=woff[:].rearrange("(t p) n -> p t n", p=128))
            offb_sb = consts.tile([1, 512], BF16)
            nc.sync.dma_start(out=offb_sb, in_=_mkap(offb[:], 0, [[0, 1], [1, 512]]))
            onesr = consts.tile([1, 128], BF16)
            nc.vector.memset(onesr, 1.0)
            load_wt(1)
            wd_sb = consts.tile([128, 6, O], BF16)
            nc.sync.dma_start(out=wd_sb, in_=wdm[:].rearrange("(t p) n -> p t n", p=128))
            ones_sb = consts.tile([128, 1], BF16)
            nc.vector.memset(ones_sb, 1.0)
            sums_sb = consts.tile([1, 1536], F32)
            # per-partition scalar constants for activation biases
            cbias = {}
            for s in (-2.0, -1.0, 0.0, 1.0, 2.0, EPS):
                cb = consts.tile([128, 1], F32, name=f"cb_{s}")
                nc.vector.memset(cb, float(s))
                cbias[s] = cb
            # warm the activation table set containing Erf (+Abs/Relu/Square)
            warm = consts.tile([128, 1], F32, name="warm")
            nc.scalar.activation(warm, cbias[0.0], AF.Erf, bias=cbias[0.0], scale=1.0)

            # BN partial sums accumulate in PSUM across all chunks
            sums_ps = ps_s.tile([1, 2048], F32, name="sums_ps")

            lams = {}
            m2s = {}
            offps = {}

            def front_end(t):
                # PE transposes of strided interior views + offsets matmul.
                # No DVE ops here.
                wt = wts[t]
                ptT = ptpool.tile([128, 6, PCH], BF16, name="ptT")
                # contiguous (c,ki,kj) patch: the PE transpose ifmap must
                # be a single-free-dim AP, so copy the strided interior
                patch = ptpool.tile([PCH, J], BF16, name="patch")
                isrc = _mkap(
                    wt, wt.offset + PAD * WIN + PAD,
                    [list(wt.ap[0]), [400, C], [WIN, 16], [1, 16]],
                )
                nc.scalar.copy(
                    out=patch.rearrange("p (c ki kj) -> p c ki kj", c=C, ki=16),
                    in_=isrc,
                )
                for q in range(6):
                    tp = ps_t.tile([128, PCH], BF16, name="tp")
                    nc.tensor.transpose(tp, patch[:, bass.ts(q, 128)], ident_sb)
                    nc.scalar.copy(out=ptT[:, q, :], in_=tp)
                offp = ps_off.tile([PCH, 512], F32, name="offp")
                for q in range(6):
                    nc.tensor.matmul(
                        offp, lhsT=ptT[:, q, :], rhs=woff_sb[:, q, :],
                        start=(q == 0), stop=False,
                    )
                # + offset bias via ones-row outer product
                nc.tensor.matmul(offp, lhsT=onesr, rhs=offb_sb,
                                 start=False, stop=True)
                offps[t] = offp

            def mid_end(t):
                # hats on ScalarE, reading the offsets straight from PSUM
                dyx = offps.pop(t)
                lam = lpool.tile([PCH, 5, 512], BF16, name="lam")
                for i, s in enumerate((-2, -1, 0, 1, 2)):
                    ab = lpool.tile([PCH, 512], BF16, name="ab")
                    nc.scalar.activation(ab, dyx, AF.Abs,
                                         bias=cbias[float(-s)], scale=1.0)
                    nc.scalar.activation(lam[:, i, :], ab, AF.Relu,
                                         bias=cbias[1.0], scale=-1.0)
                lams[t] = lam

            def emit_m2d(t):
                # m2[p, sy, sx, k] = lam_y[p, sy, k] * lam_x[p, sx, k],
                # sy rows 0..2 (DVE half)
                lam = lams[t]
                m2 = m2s[t] = mpool.tile([PCH, 25, 256], BF16, name="m2")
                m2o = _mkap(m2, m2.offset, [list(m2.ap[0]), [1280, 3], [256, 5], [1, 256]])
                lyv = _mkap(lam, lam.offset, [list(lam.ap[0]), [512, 3], [0, 5], [1, 256]])
                lxv = _mkap(lam, lam.offset + 256, [list(lam.ap[0]), [0, 3], [512, 5], [1, 256]])
                nc.vector.tensor_mul(m2o, lyv, lxv)

            def emit_m2p(t):
                # sy rows 3..4 (Pool half)
                lam, m2 = lams[t], m2s[t]
                m2o = _mkap(m2, m2.offset + 3 * 1280, [list(m2.ap[0]), [1280, 2], [256, 5], [1, 256]])
                lyv = _mkap(lam, lam.offset + 3 * 512, [list(lam.ap[0]), [512, 2], [0, 5], [1, 256]])
                lxv = _mkap(lam, lam.offset + 256, [list(lam.ap[0]), [0, 2], [512, 5], [1, 256]])
                nc.gpsimd.tensor_mul(m2o, lyv, lxv)

            TAPS_D = [tap for tap in TAPS if tap[0] <= 0]   # 13 taps
            TAPS_P = [tap for tap in TAPS if tap[0] > 0]    # 8 taps

            front_end(0)
            mid_end(0)
            front_end(1)
            mid_end(1)
            emit_m2d(0)
            emit_m2p(0)

            ystash = []
            # ================= phase A (software-pipelined, depth 2) =========
            for t in range(NCHUNK):
                if t + 2 < NCHUNK:
                    load_wt(t + 2)
                    front_end(t + 2)

                wt = wts[t]
                m2 = m2s[t]
                # tap MAC: acc[p,c,ki,kj] += m2_s[p,ki,kj] * win[p,c,ki+2+sy,kj+2+sx]
                accD = apool.tile([PCH, 768], BF16, name="accD")
                accP = apool.tile([PCH, 768], BF16, name="accP")
                for on_d, taps in ((True, TAPS_D), (False, TAPS_P)):
                    eng = nc.vector if on_d else nc.gpsimd
                    acc = accD if on_d else accP
                    av = acc.rearrange("p (c ki kj) -> p c ki kj", c=C, ki=16)
                    for i, (sy, sx) in enumerate(taps):
                        xs = _mkap(
                            wt, wt.offset + (PAD + sy) * WIN + (PAD + sx),
                            [list(wt.ap[0]), [400, C], [WIN, 16], [1, 16]],
                        )
                        mi = (sy + 2) * 5 + (sx + 2)
                        ms = _mkap(
                            m2, m2.offset + mi * 256,
                            [list(m2.ap[0]), [0, C], [16, 16], [1, 16]],
                        )
                        if i == 0:
                            eng.tensor_mul(av, xs, ms)
                        else:
                            tmp = tpool.tile([PCH, 768], BF16,
                                             name="tmpD" if on_d else "tmpP")
                            tv = tmp.rearrange("p (c ki kj) -> p c ki kj", c=C, ki=16)
                            eng.tensor_mul(tv, xs, ms)
                            eng.tensor_add(acc, acc, tmp)
                if t + 1 < NCHUNK:
                    emit_m2d(t + 1)
                    emit_m2p(t + 1)

                nc.vector.tensor_add(accD, accD, accP)
                # sampledT via PE transposes
                sT = stpool.tile([128, 6, PCH], BF16, name="sT")
                for q in range(6):
                    tp2 = ps_t.tile([128, PCH], BF16, name="tp")
                    nc.tensor.transpose(tp2, accD[:, bass.ts(q, 128)], ident_sb)
                    nc.scalar.copy(out=sT[:, q, :], in_=tp2)

                # main matmul: y [128, 768] in two PSUM halves
                y = ypool.tile([PCH, O], BF16, name="y")
                for half in range(2):
                    yp = ps_y.tile([PCH, 384], F32, name="yp")
                    for q in range(6):
                        nc.tensor.matmul(
                            yp, lhsT=sT[:, q, :],
                            rhs=wd_sb[:, q, bass.ts(half, 384)],
                            start=(q == 0), stop=(q == 5),
                        )
                    nc.scalar.copy(out=y[:, bass.ts(half, 384)], in_=yp)
                ystash.append(y)

                # BN partial sums accumulate in PSUM (pad rows have y == 0)
                ysq = sqpool.tile([PCH, O], BF16, name="ysq")
                nc.scalar.activation(ysq, y, AF.Square, bias=cbias[0.0], scale=1.0)
                for seg in range(4):
                    srcseg = (y if seg < 2 else ysq)[:, bass.ts(seg % 2, 384)]
                    nc.tensor.matmul(
                        sums_ps[:, seg * 512: seg * 512 + 384],
                        lhsT=ones_sb, rhs=srcseg,
                        start=(t == 0), stop=(t == NCHUNK - 1),
                    )
                if t + 2 < NCHUNK:
                    mid_end(t + 2)
            nc.scalar.activation(warm, cbias[0.0], AF.Sqrt, bias=cbias[EPS], scale=1.0)
            sums_v = _mkap(sums_ps, sums_ps.offset, [list(sums_ps.ap[0]), [512, 4], [1, 384]])
            nc.scalar.copy(
                out=_mkap(sums_sb, sums_sb.offset, [list(sums_sb.ap[0]), [384, 4], [1, 384]]),
                in_=sums_v)

            # ================= phase B: global BN stats =================
            cc_in = drampool.tile([1, 1536], F32, name="cc_in")
            cc_out = drampool.tile([1, 1536], F32, name="cc_out", addr_space="Shared")
            nc.sync.dma_start(out=cc_in, in_=sums_sb)
            gam = fpool.tile([128, O], F32, name="gam")
            nc.sync.dma_start(out=gam, in_=_mkap(bng[:], 0, [[0, 128], [1, O]]))
            bet = fpool.tile([128, O], F32, name="bet")
            nc.sync.dma_start(out=bet, in_=_mkap(bnb[:], 0, [[0, 128], [1, O]]))
            nc.gpsimd.collective_compute(
                "AllReduce", mybir.AluOpType.add,
                replica_groups=[list(range(n_cores))],
                ins=[cc_in.opt()], outs=[cc_out.opt()],
            )
            gsums = fpool.tile([128, 1536], F32)
            nc.sync.dma_start(out=gsums, in_=_mkap(cc_out, cc_out.offset, [[0, 128], [1, 1536]]))
            ascb = fpool.tile([128, O], BF16, name="ascb")
            bshb = fpool.tile([128, O], BF16, name="bshb")

            def dp_split(fn):
                # run an elementwise [128, 768] step as two half-width ops,
                # DVE on [0:384], Pool on [384:768]
                fn(nc.vector, slice(0, 384))
                fn(nc.gpsimd, slice(384, 768))

            mean = fpool.tile([128, O], F32, name="ftmp", tag="ftmp", bufs=3)
            dp_split(lambda e, s: e.tensor_scalar_mul(mean[:, s], gsums[:, 0:768][:, s], 1.0 / NTOT))
            var = fpool.tile([128, O], F32, name="ftmp2", tag="ftmp", bufs=3)
            dp_split(lambda e, s: e.tensor_mul(var[:, s], mean[:, s], mean[:, s]))
            # var = S2/N - mean^2 in one fused op (TensorScalarPtr is DVE-only)
            nc.vector.scalar_tensor_tensor(
                var, gsums[:, 768:1536], 1.0 / NTOT, var,
                mybir.AluOpType.mult, mybir.AluOpType.subtract)
            # rstd = 1/sqrt(var + eps): ScalarE sqrt + fast DVE reciprocal
            sd = fpool.tile([128, O], F32, name="ftmp3", tag="ftmp", bufs=3)
            nc.scalar.activation(sd, var, AF.Sqrt, bias=cbias[EPS], scale=1.0)
            rstd = fpool.tile([128, O], F32, name="ftmp4", tag="ftmp", bufs=3)
            nc.vector.reciprocal_approx_fast(rstd, sd)
            # asc = (gamma/2)*rstd ; bsh = beta/2 - mean*asc (GELU 0.5
            # pre-folded into bng/bnb on the host)
            dp_split(lambda e, s: e.tensor_mul(ascb[:, s], gam[:, s], rstd[:, s]))
            bsh = fpool.tile([128, O], F32, name="bsh")
            dp_split(lambda e, s: e.tensor_mul(bsh[:, s], mean[:, s], ascb[:, s]))
            dp_split(lambda e, s: e.tensor_sub(bshb[:, s], bet[:, s], bsh[:, s]))

            # ================= phase C: normalize + GELU + store =================
            yms = {}

            def emit_ym(t):
                ym = cpool.tile([PCH, O], BF16, name="ym", bufs=4)
                nc.vector.tensor_mul(ym, ystash[t], ascb)
                yms[t] = ym

            emit_ym(0)
            emit_ym(1)
            for t in range(NCHUNK):
                if t + 2 < NCHUNK:
                    emit_ym(t + 2)
                ym = yms.pop(t)
                yn = cpool.tile([PCH, O], BF16, name="yn", bufs=4)
                nc.gpsimd.tensor_add(yn, ym, bshb)
                g = cpool.tile([PCH, O], BF16, name="g", bufs=4)
                # yn = 0.5*(BN affine); gelu = (erf(yn*2/sqrt2)+1)*yn
                nc.scalar.activation(g, yn, AF.Erf, bias=cbias[0.0], scale=SQRT2)
                gout = gpool.tile([PCH, O], F32, name="gout")
                nc.vector.scalar_tensor_tensor(
                    gout, g, 1.0, yn, mybir.AluOpType.add, mybir.AluOpType.mult
                )
                nrows = min(PCH, NPOS - t * PCH)
                nc.sync.dma_start(
                    out=outd[t * PCH: t * PCH + nrows, :],
                    in_=gout[:nrows, :],
                )

    nc.compile()
    return nc


def _host_prep(x, offset_w, offset_b, dconv_w):
    x = np.asarray(x, np.float32)
    xpad = np.zeros((B, C, H + 2 * PAD, W + 2 * PAD), np.float32)
    xpad[:, :, PAD:PAD + H, PAD:PAD + W] = x
    sb, sc, sy, sx = xpad.strides
    # windows c-major: [B, ho, wo, c, wi, wj]
    win6 = np.lib.stride_tricks.as_strided(
        xpad, shape=(B, HO, WO, C, WIN, WIN),
        strides=(sb, 16 * sy, 16 * sx, sc, sy, sx),
    )
    xwin = np.ascontiguousarray(win6).reshape(B, HO * WO, NWIN).astype(BF)

    # weights to flat-j (c, ki, kj) order
    woff = np.asarray(offset_w, np.float32).transpose(1, 2, 3, 0).reshape(J, 512)
    perm = np.r_[np.arange(0, 512, 2), np.arange(1, 512, 2)]
    woff = np.ascontiguousarray(woff[:, perm]).astype(BF)
    offbp = np.ascontiguousarray(np.asarray(offset_b, np.float32)[perm]).astype(BF)
    wd = np.ascontiguousarray(
        np.asarray(dconv_w, np.float32).transpose(1, 2, 3, 0).reshape(J, O)
    ).astype(BF)
    return xwin, woff, offbp, wd


def _in_maps(x, offset_w, offset_b, dconv_w, bn_gamma, bn_beta):
    xwin, woff, offbp, wd = _host_prep(x, offset_w, offset_b, dconv_w)
    ident = np.eye(128, dtype=BF)
    bngk = 0.5 * np.asarray(bn_gamma, np.float32)
    bnbk = 0.5 * np.asarray(bn_beta, np.float32)
    in_maps = []
    for c in range(NCORES):
        xc = xwin[c * BL:(c + 1) * BL].reshape(NPOS, NWIN)
        xc_pad = np.zeros((NPAD, NWIN), BF)
        xc_pad[:NPOS] = xc
        in_maps.append({
            "xwin": np.ascontiguousarray(xc_pad.reshape(NCHUNK, PCH, NWIN)),
            "woff": woff, "wdm": wd, "offb": offbp,
            "bng": bngk, "bnb": bnbk, "ident": ident,
        })
    return in_maps


def kernel(x, offset_w, offset_b, dconv_w, bn_gamma, bn_beta):
    if "nc" not in _CACHE:
        _CACHE["nc"] = _build()
    nc = _CACHE["nc"]
    in_maps = _in_maps(x, offset_w, offset_b, dconv_w, bn_gamma, bn_beta)
    res = run_bass_kernel_spmd(nc, in_maps, list(range(NCORES)))
    outs = [res.results[c]["out"][:NPOS] for c in range(NCORES)]
    return np.concatenate(outs, axis=0).reshape(B, HO * WO, O).astype(np.float32)


if __name__ == "__main__":
    _build()
    print("build ok")


# revision 27
# speedup vs baseline: 1.2241x; 1.0969x over previous
# Trainium2 Bass kernel for nn_DeformablePatchEmbed_GELU (deformable patch
# embed + BatchNorm(batch stats) + exact GELU), data-parallel over 8 cores.
#
# v2: bf16 datapath. Per core, 1568 positions (8 images x 14x14) packed as
# 13 chunks of 128 partition-rows (last chunk 32 real rows, zero-padded).
# Windows are stored c-major [c, wi, wj] (20x20x3) in bf16 so every DVE
# tensor op has a packed (stride-1) innermost dim -> 2x DVE rate, and the
# PE matmuls/transposes run 1-pass bf16 (~5x faster than fp32).
#
# Per chunk:
#   - one DMA loads win [128, 1200] bf16
#   - PE transposes 6 strided 128-column slices of the interior patch
#     (flat (c,ki,kj)) -> patchT; matmul with offset weights -> offsets
#   - hats Hat(u)=relu(1-|u|) on ScalarE (per-partition bias = -s)
#   - one DVE op forms all 25 tap products m2[sy,sx,k]=haty*hatx
#   - bilinear = sum_s m2_s * win_shift_s: mul+add chains split between
#     VectorE (bf16 2x) and GpSimd, two independent accumulators
#   - PE transposes acc -> sampledT; matmul with dconv weights -> y
#   - BN partial sums via ones-matmuls, accumulated into SBUF
# AllReduce (8 cores) of 1536 sums -> BN scale/shift (folded w/ GELU 0.5);
# phase C: normalize + exact GELU (Erf LUT) + store fp32.
import numpy as np
import ml_dtypes

import concourse.bacc as bacc
import concourse.bass as bass
import concourse.tile as tile
from concourse import mybir
from concourse.bass_utils import run_bass_kernel_spmd

F32 = mybir.dt.float32
BF16 = mybir.dt.bfloat16
AF = mybir.ActivationFunctionType
BF = ml_dtypes.bfloat16

# problem dims (hardcoded per contract)
B, C, H, W = 64, 3, 224, 224
O = 768
PATCH = 16
NCORES = 8
BL = B // NCORES            # 8 images per core
HO = WO = 14
NPOS = BL * HO * WO         # 1568 positions per core
PCH = 128                   # positions per chunk (partition rows)
NCHUNK = 13                 # ceil(1568/128); last chunk has 32 real rows
NPAD = NCHUNK * PCH         # 1664
PAD = 2
J = 768                     # patch flat size (c,ki,kj)
NTOT = float(B * HO * WO)   # 12544 positions globally (BN denominator)
EPS = 1e-5
WIN = 20                    # window side
NWIN = WIN * WIN * C        # 1200, stored c-major: idx = c*400 + wi*20 + wj
SQRT2 = 1.4142135623730951

# tap split: first TD taps on DVE, rest on GpSimd(Pool)
TAPS = [(sy, sx) for sy in range(-2, 3) for sx in range(-2, 3)
        if not (abs(sy) == 2 and abs(sx) == 2)]
TD = 11

_CACHE = {}


def _mkap(handle_ap, offset, dims):
    return bass.AP(tensor=handle_ap.tensor, offset=offset, ap=[list(d) for d in dims])


def _build(n_cores=NCORES):
    nc = bacc.Bacc("TRN2", target_bir_lowering=False, debug=False, num_devices=n_cores)
    xwin = nc.dram_tensor("xwin", [NCHUNK, PCH, NWIN], BF16, kind="ExternalInput")
    woff = nc.dram_tensor("woff", [J, 512], BF16, kind="ExternalInput")
    wdm = nc.dram_tensor("wdm", [J, O], BF16, kind="ExternalInput")
    offb = nc.dram_tensor("offb", [512], BF16, kind="ExternalInput")
    bng = nc.dram_tensor("bng", [O], F32, kind="ExternalInput")
    bnb = nc.dram_tensor("bnb", [O], F32, kind="ExternalInput")
    ident = nc.dram_tensor("ident", [128, 128], BF16, kind="ExternalInput")
    outd = nc.dram_tensor("out", [NPAD, O], F32, kind="ExternalOutput")

    from contextlib import ExitStack
    with tile.TileContext(nc) as tc:
        with ExitStack() as ctx:
            consts = ctx.enter_context(tc.tile_pool(name="consts", bufs=1))
            wpool = ctx.enter_context(tc.tile_pool(name="wpool", bufs=3))
            ptpool = ctx.enter_context(tc.tile_pool(name="ptpool", bufs=2))
            dpool = ctx.enter_context(tc.tile_pool(name="dpool", bufs=2))
            lpool = ctx.enter_context(tc.tile_pool(name="lpool", bufs=2))
            mpool = ctx.enter_context(tc.tile_pool(name="mpool", bufs=2))
            apool = ctx.enter_context(tc.tile_pool(name="apool", bufs=2))
            tpool = ctx.enter_context(tc.tile_pool(name="tpool", bufs=2))
            stpool = ctx.enter_context(tc.tile_pool(name="stpool", bufs=2))
            ypool = ctx.enter_context(tc.tile_pool(name="ypool", bufs=NCHUNK))
            sqpool = ctx.enter_context(tc.tile_pool(name="sqpool", bufs=2))
            cpool = ctx.enter_context(tc.tile_pool(name="cpool", bufs=4))
            gpool = ctx.enter_context(tc.tile_pool(name="gpool", bufs=4))
            fpool = ctx.enter_context(tc.tile_pool(name="fpool", bufs=1))
            ps_t = ctx.enter_context(tc.tile_pool(name="ps_t", bufs=2, space="PSUM"))
            ps_off = ctx.enter_context(tc.tile_pool(name="ps_off", bufs=1, space="PSUM"))
            ps_y = ctx.enter_context(tc.tile_pool(name="ps_y", bufs=1, space="PSUM"))
            ps_s = ctx.enter_context(tc.tile_pool(name="ps_s", bufs=1, space="PSUM"))
            drampool = ctx.enter_context(tc.tile_pool(name="dram", bufs=1, space="DRAM"))

            # ---- constants (ordered so chunk-0 work starts early) ----
            ident_sb = consts.tile([128, 128], BF16)
            nc.sync.dma_start(out=ident_sb, in_=ident[:])

            wts = {}

            def load_wt(t):
                w = wpool.tile([PCH, NWIN], BF16, name="wt")
                nc.sync.dma_start(out=w, in_=xwin[t])
                wts[t] = w

            load_wt(0)
            woff_sb = consts.tile([128, 6, 512], BF16)
            nc.sync.dma_start(out=woff_sb, in_=woff[:].rearrange("(t p) n -> p t n", p=128))
            offb_sb = consts.tile([1, 512], BF16)
            nc.sync.dma_start(out=offb_sb, in_=_mkap(offb[:], 0, [[0, 1], [1, 512]]))
            onesr = consts.tile([1, 128], BF16)
            nc.vector.memset(onesr, 1.0)
            load_wt(1)
            wd_sb = consts.tile([128, 6, O], BF16)
            nc.sync.dma_start(out=wd_sb, in_=wdm[:].rearrange("(t p) n -> p t n", p=128))
            ones_sb = consts.tile([128, 1], BF16)
            nc.vector.memset(ones_sb, 1.0)
            sums_sb = consts.tile([1, 1536], F32)
            # per-partition scalar constants for activation biases
            cbias = {}
            for s in (-2.0, -1.0, 0.0, 1.0, 2.0, EPS):
                cb = consts.tile([128, 1], F32, name=f"cb_{s}")
                nc.vector.memset(cb, float(s))
                cbias[s] = cb
            # warm the activation table set containing Erf (+Abs/Relu/Square)
            warm = consts.tile([128, 1], F32, name="warm")
            nc.scalar.activation(warm, cbias[0.0], AF.Erf, bias=cbias[0.0], scale=1.0)

            # BN partial sums accumulate in PSUM across all chunks
            sums_ps = ps_s.tile([1, 2048], F32, name="sums_ps")

            lams = {}
            m2s = {}
            offps = {}

            def front_end(t):
                # PE transposes of strided interior views + offsets matmul.
                # No DVE ops here.
                wt = wts[t]
                ptT = ptpool.tile([128, 6, PCH], BF16, name="ptT")
                # contiguous (c,ki,kj) patch: the PE transpose ifmap must
                # be a single-free-dim AP, so copy the strided interior
                patch = ptpool.tile([PCH, J], BF16, name="patch")
                isrc = _mkap(
                    wt, wt.offset + PAD * WIN + PAD,
                    [list(wt.ap[0]), [400, C], [WIN, 16], [1, 16]],
                )
                nc.scalar.copy(
                    out=patch.rearrange("p (c ki kj) -> p c ki kj", c=C, ki=16),
                    in_=isrc,
                )
                for q in range(6):
                    tp = ps_t.tile([128, PCH], BF16, name="tp")
                    nc.tensor.transpose(tp, patch[:, bass.ts(q, 128)], ident_sb)
                    nc.scalar.copy(out=ptT[:, q, :], in_=tp)
                offp = ps_off.tile([PCH, 512], F32, name="offp")
                for q in range(6):
                    nc.tensor.matmul(
                        offp, lhsT=ptT[:, q, :], rhs=woff_sb[:, q, :],
                        start=(q == 0), stop=False,
                    )
                # + offset bias via ones-row outer product
                nc.tensor.matmul(offp, lhsT=onesr, rhs=offb_sb,
                                 start=False, stop=True)
                offps[t] = offp

            def mid_end(t):
                # hats on ScalarE, reading the offsets straight from PSUM
                dyx = offps.pop(t)
                lam = lpool.tile([PCH, 5, 512], BF16, name="lam")
                for i, s in enumerate((-2, -1, 0, 1, 2)):
                    ab = lpool.tile([PCH, 512], BF16, name="ab")
                    nc.scalar.activation(ab, dyx, AF.Abs,
                                         bias=cbias[float(-s)], scale=1.0)
                    nc.scalar.activation(lam[:, i, :], ab, AF.Relu,
                                         bias=cbias[1.0], scale=-1.0)
                lams[t] = lam

            def emit_m2d(t):
                # m2[p, sy, sx, k] = lam_y[p, sy, k] * lam_x[p, sx, k],
                # sy rows 0..2 (DVE half)
                lam = lams[t]
                m2 = m2s[t] = mpool.tile([PCH, 25, 256], BF16, name="m2")
                m2o = _mkap(m2, m2.offset, [list(m2.ap[0]), [1280, 3], [256, 5], [1, 256]])
                lyv = _mkap(lam, lam.offset, [list(lam.ap[0]), [512, 3], [0, 5], [1, 256]])
                lxv = _mkap(lam, lam.offset + 256, [list(lam.ap[0]), [0, 3], [512, 5], [1, 256]])
                nc.vector.tensor_mul(m2o, lyv, lxv)

            def emit_m2p(t):
                # sy rows 3..4 (Pool half)
                lam, m2 = lams[t], m2s[t]
                m2o = _mkap(m2, m2.offset + 3 * 1280, [list(m2.ap[0]), [1280, 2], [256, 5], [1, 256]])
                lyv = _mkap(lam, lam.offset + 3 * 512, [list(lam.ap[0]), [512, 2], [0, 5], [1, 256]])
                lxv = _mkap(lam, lam.offset + 256, [list(lam.ap[0]), [0, 2], [512, 5], [1, 256]])
                nc.gpsimd.tensor_mul(m2o, lyv, lxv)

            TAPS_D = [tap for tap in TAPS if tap[0] <= 0]   # 13 taps
            TAPS_P = [tap for tap in TAPS if tap[0] > 0]    # 8 taps

            front_end(0)
            mid_end(0)
            front_end(1)
            mid_end(1)
            emit_m2d(0)
            emit_m2p(0)

            ystash = []
            # ================= phase A (software-pipelined, depth 2) =========
            for t in range(NCHUNK):
                if t + 2 < NCHUNK:
                    load_wt(t + 2)
                    front_end(t + 2)

                wt = wts[t]
                m2 = m2s[t]
                # tap MAC: acc[p,c,ki,kj] += m2_s[p,ki,kj] * win[p,c,ki+2+sy,kj+2+sx]
                accD = apool.tile([PCH, 768], BF16, name="accD")
                accP = apool.tile([PCH, 768], BF16, name="accP")
                for on_d, taps in ((True, TAPS_D), (False, TAPS_P)):
                    eng = nc.vector if on_d else nc.gpsimd
                    acc = accD if on_d else accP
                    av = acc.rearrange("p (c ki kj) -> p c ki kj", c=C, ki=16)
                    for i, (sy, sx) in enumerate(taps):
                        xs = _mkap(
                            wt, wt.offset + (PAD + sy) * WIN + (PAD + sx),
                            [list(wt.ap[0]), [400, C], [WIN, 16], [1, 16]],
                        )
                        mi = (sy + 2) * 5 + (sx + 2)
                        ms = _mkap(
                            m2, m2.offset + mi * 256,
                            [list(m2.ap[0]), [0, C], [16, 16], [1, 16]],
                        )
                        if i == 0:
                            eng.tensor_mul(av, xs, ms)
                        else:
                            tmp = tpool.tile([PCH, 768], BF16,
                                             name="tmpD" if on_d else "tmpP")
                            tv = tmp.rearrange("p (c ki kj) -> p c ki kj", c=C, ki=16)
                            eng.tensor_mul(tv, xs, ms)
                            eng.tensor_add(acc, acc, tmp)
                if t + 1 < NCHUNK:
                    emit_m2d(t + 1)
                    emit_m2p(t + 1)

                nc.gpsimd.tensor_add(accD, accD, accP)
                # sampledT via PE transposes
                sT = stpool.tile([128, 6, PCH], BF16, name="sT")
                for q in range(6):
                    tp2 = ps_t.tile([128, PCH], BF16, name="tp")
                    nc.tensor.transpose(tp2, accD[:, bass.ts(q, 128)], ident_sb)
                    nc.scalar.copy(out=sT[:, q, :], in_=tp2)

                # main matmul: y [128, 768] in two PSUM halves
                y = ypool.tile([PCH, O], BF16, name="y")
                for half in range(2):
                    yp = ps_y.tile([PCH, 384], F32, name="yp")
                    for q in range(6):
                        nc.tensor.matmul(
                            yp, lhsT=sT[:, q, :],
                            rhs=wd_sb[:, q, bass.ts(half, 384)],
                            start=(q == 0), stop=(q == 5),
                        )
                    nc.scalar.copy(out=y[:, bass.ts(half, 384)], in_=yp)
                ystash.append(y)

                # BN partial sums accumulate in PSUM (pad rows have y == 0)
                ysq = sqpool.tile([PCH, O], BF16, name="ysq")
                nc.scalar.activation(ysq, y, AF.Square, bias=cbias[0.0], scale=1.0)
                for seg in range(4):
                    srcseg = (y if seg < 2 else ysq)[:, bass.ts(seg % 2, 384)]
                    nc.tensor.matmul(
                        sums_ps[:, seg * 512: seg * 512 + 384],
                        lhsT=ones_sb, rhs=srcseg,
                        start=(t == 0), stop=(t == NCHUNK - 1),
                    )
                if t + 2 < NCHUNK:
                    mid_end(t + 2)
            nc.scalar.activation(warm, cbias[0.0], AF.Sqrt, bias=cbias[EPS], scale=1.0)
            sums_v = _mkap(sums_ps, sums_ps.offset, [list(sums_ps.ap[0]), [512, 4], [1, 384]])
            nc.scalar.copy(
                out=_mkap(sums_sb, sums_sb.offset, [list(sums_sb.ap[0]), [384, 4], [1, 384]]),
                in_=sums_v)

            # ================= phase B: global BN stats =================
            cc_in = drampool.tile([1, 1536], F32, name="cc_in")
            cc_out = drampool.tile([1, 1536], F32, name="cc_out", addr_space="Shared")
            nc.sync.dma_start(out=cc_in, in_=sums_sb)
            gam = fpool.tile([128, O], F32, name="gam")
            nc.sync.dma_start(out=gam, in_=_mkap(bng[:], 0, [[0, 128], [1, O]]))
            bet = fpool.tile([128, O], F32, name="bet")
            nc.sync.dma_start(out=bet, in_=_mkap(bnb[:], 0, [[0, 128], [1, O]]))
            nc.gpsimd.collective_compute(
                "AllReduce", mybir.AluOpType.add,
                replica_groups=[list(range(n_cores))],
                ins=[cc_in.opt()], outs=[cc_out.opt()],
            )
            gsums = fpool.tile([128, 1536], F32)
            nc.sync.dma_start(out=gsums, in_=_mkap(cc_out, cc_out.offset, [[0, 128], [1, 1536]]))
            ascb = fpool.tile([128, O], BF16, name="ascb")
            bshb = fpool.tile([128, O], BF16, name="bshb")

            def dp_split(fn):
                # run an elementwise [128, 768] step as two half-width ops,
                # DVE on [0:384], Pool on [384:768]
                fn(nc.vector, slice(0, 384))
                fn(nc.gpsimd, slice(384, 768))

            mean = fpool.tile([128, O], F32, name="ftmp", tag="ftmp", bufs=3)
            dp_split(lambda e, s: e.tensor_scalar_mul(mean[:, s], gsums[:, 0:768][:, s], 1.0 / NTOT))
            var = fpool.tile([128, O], F32, name="ftmp2", tag="ftmp", bufs=3)
            dp_split(lambda e, s: e.tensor_mul(var[:, s], mean[:, s], mean[:, s]))
            # var = S2/N - mean^2 in one fused op (TensorScalarPtr is DVE-only)
            nc.vector.scalar_tensor_tensor(
                var, gsums[:, 768:1536], 1.0 / NTOT, var,
                mybir.AluOpType.mult, mybir.AluOpType.subtract)
            # rstd = 1/sqrt(var + eps): ScalarE sqrt + fast DVE reciprocal
            sd = fpool.tile([128, O], F32, name="ftmp3", tag="ftmp", bufs=3)
            nc.scalar.activation(sd, var, AF.Sqrt, bias=cbias[EPS], scale=1.0)
            rstd = fpool.tile([128, O], F32, name="ftmp4", tag="ftmp", bufs=3)
            nc.vector.reciprocal_approx_fast(rstd, sd)
            # asc = (gamma/2)*rstd ; bsh = beta/2 - mean*asc (GELU 0.5
            # pre-folded into bng/bnb on the host)
            dp_split(lambda e, s: e.tensor_mul(ascb[:, s], gam[:, s], rstd[:, s]))
            bsh = fpool.tile([128, O], F32, name="bsh")
            dp_split(lambda e, s: e.tensor_mul(bsh[:, s], mean[:, s], ascb[:, s]))
            dp_split(lambda e, s: e.tensor_sub(bshb[:, s], bet[:, s], bsh[:, s]))

            # ================= phase C: normalize + GELU + store =================
            yms = {}

            def emit_ym(t):
                ym = cpool.tile([PCH, O], BF16, name="ym", bufs=4)
                nc.vector.tensor_mul(ym, ystash[t], ascb)
                yms[t] = ym

            emit_ym(0)
            emit_ym(1)
            for t in range(NCHUNK):
                if t + 2 < NCHUNK:
                    emit_ym(t + 2)
                ym = yms.pop(t)
                yn = cpool.tile([PCH, O], BF16, name="yn", bufs=4)
                nc.gpsimd.tensor_add(yn, ym, bshb)
                g = cpool.tile([PCH, O], BF16, name="g", bufs=4)
                # yn = 0.5*(BN affine); gelu = (erf(yn*2/sqrt2)+1)*yn
                nc.scalar.activation(g, yn, AF.Erf, bias=cbias[0.0], scale=SQRT2)
                gout = gpool.tile([PCH, O], F32, name="gout")
                nc.vector.scalar_tensor_tensor(
                    gout, g, 1.0, yn, mybir.AluOpType.add, mybir.AluOpType.mult
                )
                nrows = min(PCH, NPOS - t * PCH)
                nc.sync.dma_start(
                    out=outd[t * PCH: t * PCH + nrows, :],
                    in_=gout[:nrows, :],
                )

    nc.compile()
    return nc


def _host_prep(x, offset_w, offset_b, dconv_w):
    x = np.asarray(x, np.float32)
    xpad = np.zeros((B, C, H + 2 * PAD, W + 2 * PAD), np.float32)
    xpad[:, :, PAD:PAD + H, PAD:PAD + W] = x
    sb, sc, sy, sx = xpad.strides
    # windows c-major: [B, ho, wo, c, wi, wj]
    win6 = np.lib.stride_tricks.as_strided(
        xpad, shape=(B, HO, WO, C, WIN, WIN),
        strides=(sb, 16 * sy, 16 * sx, sc, sy, sx),
    )
    xwin = np.ascontiguousarray(win6).reshape(B, HO * WO, NWIN).astype(BF)

    # weights to flat-j (c, ki, kj) order
    woff = np.asarray(offset_w, np.float32).transpose(1, 2, 3, 0).reshape(J, 512)
    perm = np.r_[np.arange(0, 512, 2), np.arange(1, 512, 2)]
    woff = np.ascontiguousarray(woff[:, perm]).astype(BF)
    offbp = np.ascontiguousarray(np.asarray(offset_b, np.float32)[perm]).astype(BF)
    wd = np.ascontiguousarray(
        np.asarray(dconv_w, np.float32).transpose(1, 2, 3, 0).reshape(J, O)
    ).astype(BF)
    return xwin, woff, offbp, wd


def _in_maps(x, offset_w, offset_b, dconv_w, bn_gamma, bn_beta):
    xwin, woff, offbp, wd = _host_prep(x, offset_w, offset_b, dconv_w)
    ident = np.eye(128, dtype=BF)
    bngk = 0.5 * np.asarray(bn_gamma, np.float32)
    bnbk = 0.5 * np.asarray(bn_beta, np.float32)
    in_maps = []
    for c in range(NCORES):
        xc = xwin[c * BL:(c + 1) * BL].reshape(NPOS, NWIN)
        xc_pad = np.zeros((NPAD, NWIN), BF)
        xc_pad[:NPOS] = xc
        in_maps.append({
            "xwin": np.ascontiguousarray(xc_pad.reshape(NCHUNK, PCH, NWIN)),
            "woff": woff, "wdm": wd, "offb": offbp,
            "bng": bngk, "bnb": bnbk, "ident": ident,
        })
    return in_maps


def kernel(x, offset_w, offset_b, dconv_w, bn_gamma, bn_beta):
    if "nc" not in _CACHE:
        _CACHE["nc"] = _build()
    nc = _CACHE["nc"]
    in_maps = _in_maps(x, offset_w, offset_b, dconv_w, bn_gamma, bn_beta)
    res = run_bass_kernel_spmd(nc, in_maps, list(range(NCORES)))
    outs = [res.results[c]["out"][:NPOS] for c in range(NCORES)]
    return np.concatenate(outs, axis=0).reshape(B, HO * WO, O).astype(np.float32)


if __name__ == "__main__":
    _build()
    print("build ok")
